# revision 26
# baseline (speedup 1.0000x reference)
"""CalScaleOPTAttention on 8 TRN2 NeuronCores (v2).

Sharding: heads across cores (2 heads / core, 256 channels each), with two
token-sharded cooperative stages to kill redundant DVE work:
  - x int8 quantization is sharded by tokens (each core quantizes its 256
    tokens with per-partition scales, PE-transposes, AllGathers the bf16 ints
    + fp32 scales in one collective).
  - the output projection is sharded by tokens via AllToAll: each core
    receives the full-E context slice for its 256 tokens, computes the
    per-token quant scale locally (no AllReduce), and o-projects with the
    full wo, streaming weight tiles from HBM.
Pass-2 attention computes S^T = K_mixed @ Q^T directly (scores with keys on
partitions), so exp(S^T) feeds the PV matmul with zero transposes; the
softmax denominator comes from a ones-row matmul and 1/z = exp(-ln z) on the
scalar engine (same activation table as Exp).

Collectives: AllGather(x ints + scales), AllReduce-max(k/v quant stats),
AllReduce-add(accumulated attention score, both heads), AllToAll(ctx) x2.
"""

import numpy as np
import ml_dtypes

import concourse.bass as bass
import concourse.mybir as mybir
import concourse.tile as tile
from concourse import bacc
from concourse.bass_utils import run_bass_kernel_spmd

F32 = mybir.dt.float32
F32R = mybir.dt.float32r
BF16 = mybir.dt.bfloat16
I8 = mybir.dt.int8
AX = mybir.AxisListType
OP = mybir.AluOpType
ACTF = mybir.ActivationFunctionType

NCORES = 8
T = 2048
E = 2048
H = 16
D = 128                   # head dim
HL = H // NCORES          # heads per core = 2
CH = HL * D               # channels per core = 256
TOK = T // NCORES         # tokens per core = 256
NT = T // 128             # 16 row tiles
NE = E // 128             # 16 contraction tiles
Q8 = 127.0
Q4 = 7.0
EPS = 1e-5
NEG = -1e9
RND_C = 12582912.0        # 1.5 * 2**23 round-to-int trick constant
SCALING = float(D) ** -0.5
K_TOP = T // 40           # 51
XBLK = 128 * TOK          # one e-tile block of x ints in the AG buffer
A2A = 128 * TOK           # per-shard ctx elements


def _cdiv(a, b):
    return (a + b - 1) // b


DEBUG = False


def build(causal: bool):
    nc = bacc.Bacc("TRN2", target_bir_lowering=False, debug=False,
                   num_devices=NCORES)

    def dt_in(n, s, d):
        return nc.dram_tensor(n, s, d, kind="ExternalInput").ap()

    g = {}
    g["xrows_d"] = dt_in("xrows", [TOK, E], F32)
    for w in ("wq", "wk", "wv"):
        g[w + "_d"] = dt_in(w, [E, CH], BF16)
    g["wo_d"] = dt_in("wo", [E, E], BF16)
    for v in ("swq", "swk", "swv", "qb", "kb", "vb"):
        g[v + "_d"] = dt_in(v, [CH], F32)
    g["swo_d"] = dt_in("swo", [E], F32)
    g["ob_d"] = dt_in("ob", [E], F32)
    g["tblk_d"] = dt_in("tblk", [128, 128], F32)
    g["tblkT_d"] = dt_in("tblkT", [128, 128], F32)
    g["ident_d"] = dt_in("ident", [128, 128], F32R)
    g["identb_d"] = dt_in("identb", [128, 128], BF16)
    g["rvr_d"] = dt_in("rvr", [4, 512], F32)
    if not causal:
        g["mask_d"] = dt_in("mask", [T, T], F32)
        g["maskT_d"] = dt_in("maskT", [T, T], F32)

    g["outT_d"] = nc.dram_tensor("outT", [E, TOK], F32,
                                 kind="ExternalOutput").ap()
    if DEBUG:
        for nm, shp in (("dbg_qT", [128, T]), ("dbg_kT", [128, T]),
                        ("dbg_yv", [128, CH]), ("dbg_acc", [1, T]),
                        ("dbg_ssel", [1, T]), ("dbg_ctxT", [128, T]),
                        ("dbg_xq", [128, T]), ("dbg_vqi", [128, CH]),
                        ("dbg_k2", [128, T]), ("dbg_cb", [128, NE * TOK]),
                        ("dbg_ci", [128, NE * TOK]), ("dbg_sx", [1, T])):
            g[nm] = nc.dram_tensor(nm, shp, F32, kind="ExternalOutput").ap()

    HXB = NE * XBLK // 2
    g["agxa_in"] = nc.dram_tensor("agxa_in", [HXB], I8).ap()
    g["agxa_out"] = nc.dram_tensor("agxa_out", [NCORES, HXB], I8,
                                   addr_space="Shared").ap()
    g["agxb_in"] = nc.dram_tensor("agxb_in", [HXB], I8).ap()
    g["agxb_out"] = nc.dram_tensor("agxb_out", [NCORES, HXB], I8,
                                   addr_space="Shared").ap()
    g["sx_in"] = nc.dram_tensor("sx_in", [TOK], F32).ap()
    g["sx_out"] = nc.dram_tensor("sx_out", [NCORES, TOK], F32,
                                 addr_space="Shared").ap()
    g["sxb"] = nc.dram_tensor("sxb", [T], F32).ap()
    g["kv_in"] = nc.dram_tensor("kv_in", [2, T], F32).ap()
    g["kv_out"] = nc.dram_tensor("kv_out", [2, T], F32,
                                 addr_space="Shared").ap()
    g["acc_in0"] = nc.dram_tensor("acc_in0", [T], F32).ap()
    g["acc_out0"] = nc.dram_tensor("acc_out0", [T], F32,
                                   addr_space="Shared").ap()
    g["acc_in1"] = nc.dram_tensor("acc_in1", [T], F32).ap()
    g["acc_out1"] = nc.dram_tensor("acc_out1", [T], F32,
                                   addr_space="Shared").ap()
    g["flg_b"] = nc.dram_tensor("flg_b", [T], F32).ap()
    g["svb"] = nc.dram_tensor("svb", [CH], F32).ap()
    g["sselr_b"] = nc.dram_tensor("sselr_b", [T], F32).ap()
    g["rsselr_b"] = nc.dram_tensor("rsselr_b", [T], F32).ap()
    g["a2a0_in"] = nc.dram_tensor("a2a0_in", [NCORES * A2A], F32).ap()
    g["a2a0_out"] = nc.dram_tensor("a2a0_out", [NCORES * A2A], F32).ap()
    g["a2a1_in"] = nc.dram_tensor("a2a1_in", [NCORES * A2A], F32).ap()
    g["a2a1_out"] = nc.dram_tensor("a2a1_out", [NCORES * A2A], F32).ap()
    g["scb"] = nc.dram_tensor("scb", [TOK], F32).ap()
    g["rscb"] = nc.dram_tensor("rscb", [TOK], F32).ap()
    g["rg"] = [list(range(NCORES))]

    with tile.TileContext(nc) as tc:
        _body(nc, tc, causal, g)
    nc.compile()
    return nc


def _body(nc, tc, causal, g):
    rg = g["rg"]

    def pool(name, bufs=1, space="SBUF"):
        cm = tc.tile_pool(name=name, bufs=bufs, space=space)
        return cm, cm.__enter__()

    per_cm, per = pool("per")

    tblk = per.tile([128, 128], F32, tag="tblk")
    tblkT = per.tile([128, 128], F32, tag="tblkT")
    ident = per.tile([128, 128], F32R, tag="ident")
    identb = per.tile([128, 128], BF16, tag="identb")
    nc.sync.dma_start(tblk[:], g["tblk_d"][:])
    nc.sync.dma_start(tblkT[:], g["tblkT_d"][:])
    nc.sync.dma_start(ident[:], g["ident_d"][:])
    nc.sync.dma_start(identb[:], g["identb_d"][:])
    ones128 = per.tile([128, 128], BF16, tag="ones128")
    nc.vector.memset(ones128[:], 1.0)
    rndc_pp = per.tile([128, 1], F32, tag="rndcpp")
    nc.vector.memset(rndc_pp[:], RND_C)

    def load_pp(dram_vec, tag, ncol=2, p=per):
        t_ = p.tile([128, ncol], F32, tag=tag)
        nc.sync.dma_start(t_[:], dram_vec.rearrange("(j p) -> p j", p=128))
        return t_

    swq_pp = load_pp(g["swq_d"], "swq")
    swk_pp = load_pp(g["swk_d"], "swk")
    qb_pp = load_pp(g["qb_d"], "qb")
    kb_pp = load_pp(g["kb_d"], "kb")
    swo_pp = load_pp(g["swo_d"], "swo", ncol=NE)
    ob_pp = load_pp(g["ob_d"], "ob", ncol=NE)
    qT = [per.tile([128, T], F32R, tag=f"qT{h}", name=f"qT{h}")
          for h in range(HL)]
    k2T = [per.tile([128, T], F32R, tag=f"k2T{h}", name=f"k2T{h}")
           for h in range(HL)]
    sv_pp = per.tile([128, HL], F32, tag="svpp")

    # long-lived mid tensors, stack-nested
    kx_cm, kxp = pool("kxp")
    kTx = [kxp.tile([128, T], F32, tag=f"kTx{h}", name=f"kTx{h}")
           for h in range(HL)]
    vq_cm, vqp = pool("vqp")
    vqi = vqp.tile([128, NT * CH], BF16, tag="vqi")
    yv_cm, yvp = pool("yvp")
    yv = yvp.tile([128, NT * CH], F32, tag="yv")

    k1r_cm, k1rp = pool("k1rp")
    k1r = [k1rp.tile([128, T], F32R, tag=f"k1r{h}", name=f"k1r{h}")
           for h in range(HL)]

    # -------- P0: local token-shard quantize of x, then one AllGather ------
    p0_cm, p0 = pool("p0")
    ps0_cm, ps0 = pool("ps0", bufs=4, space="PSUM")
    xr = [p0.tile([128, E], F32, tag=f"xr{j}", name=f"xr{j}") for j in range(2)]
    nc.sync.dma_start(xr[0][:], g["xrows_d"][0:128, :])
    nc.scalar.dma_start(xr[1][:], g["xrows_d"][128:256, :])
    am = p0.tile([128, 2], F32, tag="am")
    for j in range(2):
        nc.vector.tensor_reduce(am[:, j:j + 1], xr[j][:], axis=AX.X,
                                op=OP.max, apply_absolute_value=True)
    sxp = p0.tile([128, 2], F32, tag="sxp")
    nc.vector.tensor_scalar(sxp[:], am[:], EPS, 1.0 / Q8,
                            op0=OP.max, op1=OP.mult)
    rsxp = p0.tile([128, 2], F32, tag="rsxp")
    nc.vector.reciprocal(rsxp[:], sxp[:])
    xTi = p0.tile([128, NE * TOK], I8, tag="xTi")
    for j in range(2):
        xs = p0.tile([128, E], F32, tag="xs", bufs=2)
        nc.scalar.activation(xs[:], xr[j][:], ACTF.Identity,
                             bias=rndc_pp[:], scale=rsxp[:, j:j + 1])
        xi = p0.tile([128, E], BF16, tag="xi", bufs=2)
        nc.vector.tensor_scalar(xi[:], xs[:], RND_C, None, op0=OP.subtract)
        for et in range(NE):
            psx = ps0.tile([128, 128], BF16, tag="psx",
                           name=f"psx{j}_{et}")
            nc.tensor.transpose(psx[:], xi[:, et * 128:(et + 1) * 128],
                                identb[:])
            dst = xTi[:, et * TOK + j * 128: et * TOK + (j + 1) * 128]
            if et % 2 == 0:
                nc.scalar.copy(dst, psx[:])
            else:
                nc.vector.tensor_copy(dst, psx[:])
    dma3 = [nc.sync, nc.scalar, nc.gpsimd]
    HXB = NE * XBLK // 2
    for et in range(NE):
        agt = g["agxa_in"] if et < 8 else g["agxb_in"]
        base = (et % 8) * XBLK
        dma3[et % 3].dma_start(
            agt[base:base + XBLK].rearrange("(p t) -> p t", p=128),
            xTi[:, et * TOK:(et + 1) * TOK])
    nc.sync.dma_start(g["sx_in"].rearrange("(j p) -> p j", p=128), sxp[:])
    nc.gpsimd.collective_compute("AllGather", OP.bypass,
                                 ins=[g["sx_in"][:]], outs=[g["sx_out"][:]],
                                 replica_groups=rg)
    nc.gpsimd.collective_compute("AllGather", OP.bypass,
                                 ins=[g["agxa_in"][:]],
                                 outs=[g["agxa_out"][:]],
                                 replica_groups=rg)
    nc.gpsimd.collective_compute("AllGather", OP.bypass,
                                 ins=[g["agxb_in"][:]],
                                 outs=[g["agxb_out"][:]],
                                 replica_groups=rg)
    ps0_cm.__exit__(None, None, None)
    p0_cm.__exit__(None, None, None)

    # -------- w pool: xq + per-token scale reps --------
    w_cm, wp = pool("wpool")
    swv_row = wp.tile([1, CH], F32, tag="swvr")
    vb_row = wp.tile([1, CH], F32, tag="vbr")
    nc.sync.dma_start(swv_row[:], g["swv_d"].rearrange("(a c) -> a c", a=1))
    nc.sync.dma_start(vb_row[:], g["vb_d"].rearrange("(a c) -> a c", a=1))
    swv_rep = wp.tile([128, CH], F32, tag="swvrep")
    vb_rep = wp.tile([128, CH], F32, tag="vbrep")
    nc.gpsimd.partition_broadcast(swv_rep[:], swv_row[:])
    nc.gpsimd.partition_broadcast(vb_rep[:], vb_row[:])

    # -------- P1a: assemble xq + scales from the AllGather --------
    xq = wp.tile([128, NE * T], BF16, tag="xq")
    p1x_cm, p1x = pool("p1x", bufs=2)
    for et in range(NE):
        agt = g["agxa_out"] if et < 8 else g["agxb_out"]
        base = (et % 8) * XBLK
        xq8 = p1x.tile([128, T], I8, tag="xq8", bufs=2, name=f"xq8_{et}")
        for c in range(NCORES):
            dma3[(et * NCORES + c) % 3].dma_start(
                xq8[:, c * TOK:(c + 1) * TOK],
                agt[c, base:base + XBLK]
                .rearrange("(p t) -> p t", p=128))
        nc.vector.tensor_copy(xq[:, et * T:(et + 1) * T], xq8[:])
    p1x_cm.__exit__(None, None, None)
    sx_pp = wp.tile([128, 2 * NCORES], F32, tag="sxpp")
    for c in range(NCORES):
        nc.scalar.dma_start(
            sx_pp[:, c * 2:(c + 1) * 2],
            g["sx_out"][c, :].rearrange("(j p) -> p j", p=128))
    # bounce sx to DRAM row, then broadcast-load [128, T]
    nc.sync.dma_start(g["sxb"].rearrange("(j p) -> p j", p=128), sx_pp[:])
    sx_rep = wp.tile([128, T], F32, tag="sxrep")
    nc.sync.dma_start(sx_rep[:],
                      g["sxb"].rearrange("(a t) -> a t", a=1)
                      .to_broadcast([128, T]))
    if DEBUG:
        dbx = wp.tile([128, T], F32, tag="dbx")
        nc.vector.tensor_copy(dbx[:], xq[:, 0:T])
        nc.sync.dma_start(g["dbg_xq"][:], dbx[:])
        nc.sync.dma_start(g["dbg_sx"][:], sx_rep[0:1, :])

    # -------- P1b: Q/K/V projections --------
    p1b_cm, p1b = pool("p1b", bufs=3)
    ps1_cm, ps1 = pool("ps1", space="PSUM")
    for name, wd, sw_pp, b_pp in (
            ("q", g["wq_d"], swq_pp, qb_pp),
            ("k", g["wk_d"], swk_pp, kb_pp)):
        pP = [ps1.tile([128, T], F32, tag=f"pP{o}", name=f"pP{o}_{name}")
              for o in range(2)]
        for et in range(NE):
            we = p1b.tile([128, CH], BF16, tag="wstream")
            nc.sync.dma_start(we[:], wd[et * 128:(et + 1) * 128, :])
            xq_e = xq[:, et * T:(et + 1) * T]
            for o in range(2):
                for n in range(4):
                    nsl = slice(n * 512, (n + 1) * 512)
                    nc.tensor.matmul(pP[o][:, nsl],
                                     we[:, o * 128:(o + 1) * 128],
                                     xq_e[:, nsl],
                                     start=(et == 0), stop=(et == NE - 1))
        for o in range(2):
            e1 = p1b.tile([128, T], F32, tag="ev1", bufs=2)
            nc.scalar.activation(e1[:], pP[o][:], ACTF.Identity,
                                 bias=0.0, scale=sw_pp[:, o:o + 1])
            nc.vector.tensor_tensor(e1[:], e1[:], sx_rep[:], op=OP.mult)
            if name == "q":
                nc.vector.tensor_scalar(qT[o][:], e1[:], b_pp[:, o:o + 1],
                                        SCALING, op0=OP.add, op1=OP.mult)
            else:
                nc.vector.tensor_scalar(kTx[o][:], e1[:], b_pp[:, o:o + 1],
                                        None, op0=OP.add)
                nc.vector.tensor_copy(k1r[o][:], kTx[o][:])
    ps1_cm.__exit__(None, None, None)

    # V pass: [t, ch] = xq_block.T @ wv_int
    ps1v_cm, ps1v = pool("ps1v", bufs=2, space="PSUM")
    wv_res = p1b.tile([128, NE * CH], BF16, tag="wvres", bufs=1)
    for et in range(NE):
        nc.sync.dma_start(wv_res[:, et * CH:(et + 1) * CH],
                          g["wv_d"][et * 128:(et + 1) * 128, :])
    for j in range(NT):
        pVj = ps1v.tile([128, CH], F32, tag="pV", name=f"pV{j}")
        for et in range(NE):
            xq_e = xq[:, et * T:(et + 1) * T]
            nc.tensor.matmul(pVj[:],
                             xq_e[:, j * 128:(j + 1) * 128],
                             wv_res[:, et * CH:(et + 1) * CH],
                             start=(et == 0), stop=(et == NE - 1))
        ysl = slice(j * CH, (j + 1) * CH)
        e3 = p1b.tile([128, CH], F32, tag="ev3")
        nc.scalar.activation(e3[:], pVj[:], ACTF.Identity,
                             bias=0.0, scale=sx_pp[:, j:j + 1])
        nc.vector.tensor_tensor(e3[:], e3[:], swv_rep[:], op=OP.mult)
        nc.vector.tensor_tensor(yv[:, ysl], e3[:], vb_rep[:], op=OP.add)
    ps1v_cm.__exit__(None, None, None)
    p1b_cm.__exit__(None, None, None)
    w_cm.__exit__(None, None, None)

    if DEBUG:
        nc.sync.dma_start(g["dbg_qT"][:], qT[0][:].bitcast(F32))
        nc.sync.dma_start(g["dbg_kT"][:], kTx[0][:])
        nc.sync.dma_start(g["dbg_yv"][:], yv[:, 0:CH])

    # -------- P23: k row-max + v out-quant row-max, AllReduce-max --------
    p23_cm, p23 = pool("p23", bufs=2)
    ps23_cm, ps23 = pool("ps23", bufs=2, space="PSUM")
    kmm = p23.tile([128, T], F32, tag="kmm", bufs=1)
    kab = p23.tile([128, T], F32, tag="kab", bufs=1)
    nc.scalar.activation(kmm[:], kTx[0][:], ACTF.Abs)
    nc.scalar.activation(kab[:], kTx[1][:], ACTF.Abs)
    nc.vector.tensor_tensor(kmm[:], kmm[:], kab[:], op=OP.max)
    kmax_pp = p23.tile([128, NT], F32, tag="kmaxpp", bufs=1)
    for b in range(NT):
        psT = ps23.tile([128, 128], F32, tag="pstat", name=f"psk{b}")
        nc.tensor.transpose(psT[:], kmm[:, b * 128:(b + 1) * 128],
                            ident[:].bitcast(F32))
        nc.vector.tensor_reduce(kmax_pp[:, b:b + 1], psT[:],
                                axis=AX.X, op=OP.max,
                                apply_absolute_value=True)
    nc.sync.dma_start(g["kv_in"][0, :].rearrange("(j p) -> p j", p=128),
                      kmax_pp[:])
    ym = p23.tile([128, NT], F32, tag="ym")
    for j in range(NT):
        nc.vector.tensor_reduce(ym[:, j:j + 1], yv[:, j * CH:(j + 1) * CH],
                                axis=AX.X, op=OP.max,
                                apply_absolute_value=True)
    nc.sync.dma_start(g["kv_in"][1, :].rearrange("(j p) -> p j", p=128),
                      ym[:])
    nc.gpsimd.collective_compute("AllReduce", OP.max,
                                 ins=[g["kv_in"][:]], outs=[g["kv_out"][:]],
                                 replica_groups=rg)
    ps23_cm.__exit__(None, None, None)
    p23_cm.__exit__(None, None, None)

    # -------- P5: pass 1 -> accumulated attention score --------
    p5_cm, p5 = pool("p5", bufs=3)
    p5m_cm, p5m = pool("p5m", bufs=2)
    psA_cm, psA_p = pool("psA", space="PSUM")
    psS_cm, psS_p = pool("psS", bufs=2, space="PSUM")
    pA = [psA_p.tile([1, 512], F32, tag=f"pA{n}", name=f"pA{n}")
          for n in range(4)]
    accs2 = [p5.tile([1, T], F32, tag=f"accs2_{h}", bufs=1,
                     name=f"accs2_{h}") for h in range(HL)]
    for h in range(HL):
        first_wr = [True] * 4
        zgA = p5.tile([128, 4], F32, tag="zgA", bufs=2, name=f"zgA{h}")
        zgB = p5.tile([128, 4], F32, tag="zgB", bufs=2, name=f"zgB{h}")
        p1ts = {}
        for i in range(NT):
            c_cols = (i + 1) * 128 if causal else T
            nh = 1 if c_cols <= 1024 else 2
            ig = i % 4
            if not causal:
                mrow = p5m.tile([128, T], F32, tag="mrow")
                nc.sync.dma_start(mrow[:],
                                  g["mask_d"][i * 128:(i + 1) * 128, :])
            p1t = p5.tile([128, T], BF16, tag="p1t", bufs=4,
                          name=f"p1t_{h}_{i}")
            p1ts[i] = p1t
            for half in range(nh):
                base = half * 1024
                hw = min(1024, c_cols - base)
                psS = psS_p.tile([128, 1024], F32, tag="pS",
                                 name=f"pS_{h}_{i}_{half}")
                for n in range(_cdiv(hw, 512)):
                    w = min(512, hw - n * 512)
                    nc.tensor.matmul(
                        psS[:, n * 512:n * 512 + w],
                        qT[h][:, i * 128:(i + 1) * 128],
                        k1r[h][:, base + n * 512:base + n * 512 + w],
                        start=True, stop=True)
                if causal and base <= i * 128 < base + 1024:
                    doff = i * 128 - base
                    nc.vector.tensor_tensor(psS[:, doff:doff + 128],
                                            psS[:, doff:doff + 128],
                                            tblk[:], op=OP.add)
                elif not causal:
                    nc.vector.tensor_tensor(psS[:, :hw], psS[:, :hw],
                                            mrow[:, base:base + hw],
                                            op=OP.add)
                ztgt = (zgA if half == 0 else zgB)[:, ig:ig + 1]
                nc.scalar.activation(p1t[:, base:base + hw], psS[:, :hw],
                                     ACTF.Exp, bias=0.0, scale=1.0,
                                     accum_out=ztgt)
            if causal and c_cols % 512 != 0:
                pad = _cdiv(c_cols, 512) * 512
                nc.vector.memset(p1t[:, c_cols:pad], 0.0)
            if ig == 3:
                gi0 = i - 3
                if c_cols > 1024:
                    zs = p5.tile([128, 4], F32, tag="zs", bufs=2,
                                 name=f"zs{h}_{i}")
                    nc.vector.tensor_tensor(zs[:], zgA[:], zgB[:], op=OP.add)
                else:
                    zs = zgA
                rz4 = p5.tile([128, 4], BF16, tag="rz4", bufs=2,
                              name=f"rz4_{h}_{i}")
                with nc.allow_low_precision(reason="fp32r matmul feed"):
                    nc.vector.reciprocal(rz4[:], zs[:])
                for ii in range(gi0, i + 1):
                    cc = (ii + 1) * 128 if causal else T
                    for n in range(_cdiv(cc, 512)):
                        nc.tensor.matmul(
                            pA[n][:], rz4[:, ii - gi0:ii - gi0 + 1],
                            p1ts[ii][:, n * 512:(n + 1) * 512],
                            start=first_wr[n], stop=(ii == NT - 1))
                        first_wr[n] = False
                p1ts = {}
        for n in range(4):
            if n % 2 == 0:
                nc.scalar.copy(accs2[h][:, n * 512:(n + 1) * 512], pA[n][:])
            else:
                nc.vector.tensor_copy(accs2[h][:, n * 512:(n + 1) * 512],
                                      pA[n][:])
        acc_i = g["acc_in0"] if h == 0 else g["acc_in1"]
        acc_o = g["acc_out0"] if h == 0 else g["acc_out1"]
        nc.sync.dma_start(acc_i.rearrange("(a t) -> a t", a=1), accs2[h][:])
        nc.gpsimd.collective_compute("AllReduce", OP.add,
                                     ins=[acc_i[:]], outs=[acc_o[:]],
                                     replica_groups=rg)

    psS_cm.__exit__(None, None, None)
    psA_cm.__exit__(None, None, None)
    p5m_cm.__exit__(None, None, None)
    p5_cm.__exit__(None, None, None)

    k1r_cm.__exit__(None, None, None)

    # -------- P4: v quantization --------
    p4_cm, p4 = pool("p4", bufs=2)
    vq1 = p4.tile([128, NT * CH], F32, tag="vq1")
    ymf = p4.tile([128, NT], F32, tag="ymf")
    nc.sync.dma_start(ymf[:], g["kv_out"][1, :].rearrange("(j p) -> p j",
                                                          p=128))
    sy = p4.tile([128, NT], F32, tag="sy")
    nc.vector.tensor_scalar(sy[:], ymf[:], EPS, 1.0 / Q8,
                            op0=OP.max, op1=OP.mult)
    rsy = p4.tile([128, NT], F32, tag="rsy")
    nc.vector.reciprocal(rsy[:], sy[:])
    for j in range(NT):
        ysl = slice(j * CH, (j + 1) * CH)
        d1 = p4.tile([128, CH], F32, tag="d1")
        nc.scalar.activation(d1[:], yv[:, ysl], ACTF.Identity,
                             bias=rndc_pp[:], scale=rsy[:, j:j + 1])
        nc.vector.tensor_scalar(vq1[:, ysl], d1[:], RND_C, sy[:, j:j + 1],
                                op0=OP.subtract, op1=OP.mult)
    ps4_cm, ps4 = pool("ps4", bufs=2, space="PSUM")
    vmax_pp = p4.tile([128, HL], F32, tag="vmaxpp", bufs=1)
    vab = p4.tile([128, NT * CH], F32, tag="vab", bufs=1)
    for j in range(NT):
        ysl = slice(j * CH, (j + 1) * CH)
        nc.scalar.activation(vab[:, ysl], vq1[:, ysl], ACTF.Abs)
    for ob in range(HL):
        vmm = p4.tile([128, 128], F32, tag="vmm", bufs=2, name=f"vmm{ob}")
        nc.vector.tensor_copy(vmm[:], vab[:, ob * 128:(ob + 1) * 128])
        for j in range(1, NT):
            nc.vector.tensor_tensor(
                vmm[:], vmm[:],
                vab[:, j * CH + ob * 128:j * CH + (ob + 1) * 128],
                op=OP.max)
        psT = ps4.tile([128, 128], F32, tag="pstat4", name=f"psv{ob}")
        nc.tensor.transpose(psT[:], vmm[:], ident[:].bitcast(F32))
        nc.vector.tensor_reduce(vmax_pp[:, ob:ob + 1], psT[:], axis=AX.X,
                                op=OP.max, apply_absolute_value=True)
    ps4_cm.__exit__(None, None, None)
    nc.vector.tensor_scalar(sv_pp[:], vmax_pp[:], EPS, 1.0 / Q8,
                            op0=OP.max, op1=OP.mult)
    rsv_pp = p4.tile([128, HL], F32, tag="rsvpp")
    nc.vector.reciprocal(rsv_pp[:], sv_pp[:])
    nc.sync.dma_start(g["svb"].rearrange("(h p) -> p h", p=128), rsv_pp[:])
    rsv_rep = p4.tile([128, CH], F32, tag="rsvrep")
    nc.sync.dma_start(rsv_rep[:],
                      g["svb"].rearrange("(a c) -> a c", a=1)
                      .to_broadcast([128, CH]))
    for j in range(NT):
        ysl = slice(j * CH, (j + 1) * CH)
        m1 = p4.tile([128, CH], F32, tag="m1")
        nc.vector.tensor_tensor(m1[:], vq1[:, ysl], rsv_rep[:], op=OP.mult)
        m2 = p4.tile([128, CH], F32, tag="m2")
        nc.scalar.activation(m2[:], m1[:], ACTF.Identity, bias=rndc_pp[:])
        nc.vector.tensor_scalar(vqi[:, ysl], m2[:], RND_C, None,
                                op0=OP.subtract)
    p4_cm.__exit__(None, None, None)
    yv_cm.__exit__(None, None, None)

    # -------- P6: top-k flags + k scale selection + k2 ints --------
    p6_cm, p6 = pool("p6")
    acc4 = p6.tile([4, 512], F32, tag="acc4")
    acc4b = p6.tile([4, 512], F32, tag="acc4b")
    nc.sync.dma_start(acc4[:],
                      g["acc_out0"].rearrange("(r s) -> r s", r=4))
    nc.sync.dma_start(acc4b[:],
                      g["acc_out1"].rearrange("(r s) -> r s", r=4))
    nc.vector.tensor_tensor(acc4[:], acc4[:], acc4b[:], op=OP.add)
    if DEBUG:
        nc.sync.dma_start(g["dbg_acc"].rearrange("a (r s) -> (a r) s", r=4),
                          acc4[:])
    rvr = p6.tile([4, 512], F32, tag="rvr")
    nc.sync.dma_start(rvr[:], g["rvr_d"][:])
    nc.vector.tensor_tensor(acc4[:], acc4[:], rvr[:], op=OP.mult)
    nc.vector.tensor_scalar(acc4[:], acc4[:], 1.0 / H, None, op0=OP.mult)
    tkw = p6.tile([4, 512], F32, tag="tkw")
    ton = acc4[:]
    for k_on in range(0, K_TOP, 8):
        k_this = min(k_on + 8, K_TOP) - k_on
        mx8 = p6.tile([4, 8], F32, tag="mx8")
        nc.vector.max(out=mx8[:], in_=ton)
        if k_this < 8:
            nc.vector.memset(mx8[:, k_this:], 0)
        nc.vector.match_replace(out=tkw[:], in_to_replace=mx8[:],
                                in_values=ton, imm_value=0)
        ton = tkw[:]
    nc.vector.tensor_sub(out=tkw[:], in0=acc4[:], in1=tkw[:])
    nc.vector.tensor_scalar(tkw[:], tkw[:], 0.0, None, op0=OP.is_gt)
    nc.sync.dma_start(g["flg_b"].rearrange("(r s) -> r s", r=4), tkw[:])
    flags = p6.tile([1, T], F32, tag="flags")
    nc.sync.dma_start(flags[:], g["flg_b"].rearrange("(a t) -> a t", a=1))
    flags_i = p6.tile([1, T], mybir.dt.int32, tag="flagsi")
    nc.vector.tensor_scalar(flags_i[:], flags[:], 0.0, None, op0=OP.is_gt)

    kmaxf = p6.tile([1, T], F32, tag="kmaxf")
    nc.sync.dma_start(kmaxf[:], g["kv_out"][0:1, :])
    s8 = p6.tile([1, T], F32, tag="s8")
    nc.vector.tensor_scalar(s8[:], kmaxf[:], 1.0 / Q8, EPS,
                            op0=OP.mult, op1=OP.max)
    ssel = p6.tile([1, T], F32, tag="ssel")
    nc.vector.tensor_scalar(ssel[:], kmaxf[:], EPS, 1.0 / Q4,
                            op0=OP.max, op1=OP.mult)
    nc.vector.copy_predicated(ssel[:], flags_i[:], s8[:])
    rssel = p6.tile([1, T], F32, tag="rssel")
    nc.vector.reciprocal(rssel[:], ssel[:])
    ssel_rep = p6.tile([128, T], F32, tag="sselrep")
    rssel_rep = p6.tile([128, T], F32, tag="rsselrep")
    nc.gpsimd.partition_broadcast(ssel_rep[:], ssel[:])
    nc.gpsimd.partition_broadcast(rssel_rep[:], rssel[:])
    if DEBUG:
        nc.sync.dma_start(g["dbg_ssel"][:], ssel[:])
    for h in range(HL):
        kq = p6.tile([128, T], F32, tag="kq")
        nc.vector.tensor_tensor(kq[:], kTx[h][:], rssel_rep[:], op=OP.mult)
        kq2 = p6.tile([128, T], F32, tag="kq2")
        nc.scalar.activation(kq2[:], kq[:], ACTF.Identity, bias=rndc_pp[:])
        nc.vector.tensor_scalar(kq2[:], kq2[:], RND_C, None, op0=OP.subtract)
        nc.vector.tensor_tensor(k2T[h][:], kq2[:], ssel_rep[:], op=OP.mult)
    p6_cm.__exit__(None, None, None)

    # -------- P7: pass 2 attention, S^T layout --------
    ctx_cm, ctxp = pool("ctxp")
    ctxT = [ctxp.tile([128, T], F32, tag=f"ctxT{h}", name=f"ctxT{h}")
            for h in range(HL)]
    p7_cm, p7 = pool("p7", bufs=2)
    p7m_cm, p7m = pool("p7m", bufs=2)
    ps7s_cm, ps7S = pool("ps7s", space="PSUM")
    ps7c_cm, ps7C = pool("ps7c", bufs=2, space="PSUM")
    ps7z_cm, ps7Z = pool("ps7z", bufs=2, space="PSUM")
    a2a_ins = [g["a2a0_in"], g["a2a1_in"]]
    a2a_outs = [g["a2a0_out"], g["a2a1_out"]]
    for h in range(HL):
        for n in range(4):
            tbase = n * 512
            jmax = 4 * n + 3 if causal else NT - 1
            psC = ps7C.tile([128, 512], F32, tag="pC", name=f"pC{h}_{n}")
            psZ = ps7Z.tile([128, 512], F32, tag="pZ", name=f"pZ{h}_{n}")
            for jg in range(0, jmax + 1, 2):
                jn = min(2, jmax + 1 - jg)
                psSg = ps7S.tile([128, 1024], F32, tag="pSg", bufs=2,
                                 name=f"pSg{h}_{n}_{jg}")
                offs = []
                for jj in range(jn):
                    j = jg + jj
                    off = max(0, j * 128 - tbase) if causal else 0
                    offs.append(off)
                    if off > 0:
                        nc.vector.memset(psSg[:, jj * 512: jj * 512 + off],
                                         NEG)
                    nc.tensor.matmul(
                        psSg[:, jj * 512 + off: (jj + 1) * 512],
                        k2T[h][:, j * 128:(j + 1) * 128],
                        qT[h][:, tbase + off: tbase + 512],
                        start=True, stop=True)
                    if causal and tbase <= j * 128 < tbase + 512:
                        dsl = slice(jj * 512 + off, jj * 512 + off + 128)
                        nc.vector.tensor_tensor(psSg[:, dsl], psSg[:, dsl],
                                                tblkT[:], op=OP.add)
                    elif not causal:
                        mrowT = p7m.tile([128, 512], F32, tag="mrowT")
                        nc.sync.dma_start(
                            mrowT[:],
                            g["maskT_d"][j * 128:(j + 1) * 128,
                                         tbase:tbase + 512])
                        nc.vector.tensor_tensor(
                            psSg[:, jj * 512:(jj + 1) * 512],
                            psSg[:, jj * 512:(jj + 1) * 512],
                            mrowT[:], op=OP.add)
                expg = p7.tile([128, 1024], BF16, tag="expg", bufs=3,
                               name=f"expg{h}_{n}_{jg}")
                nc.scalar.activation(expg[:, :jn * 512], psSg[:, :jn * 512],
                                     ACTF.Exp, bias=0.0, scale=1.0)
                for jj in range(jn):
                    j = jg + jj
                    nc.tensor.matmul(
                        psC[:],
                        vqi[:, j * CH + h * 128: j * CH + (h + 1) * 128],
                        expg[:, jj * 512:(jj + 1) * 512],
                        start=(j == 0), stop=(j == jmax))
                for jj in range(jn):
                    j = jg + jj
                    nc.tensor.matmul(
                        psZ[:], ones128[:],
                        expg[:, jj * 512:(jj + 1) * 512],
                        start=(j == 0), stop=(j == jmax))
            rzr = p7.tile([128, 512], F32, tag="rzr")
            nc.vector.reciprocal(rzr[:], psZ[:])
            cx = p7.tile([128, 512], F32, tag="cx")
            nc.vector.tensor_tensor(cx[:], psC[:], rzr[:], op=OP.mult)
            nc.vector.tensor_scalar(ctxT[h][:, tbase:tbase + 512], cx[:],
                                    sv_pp[:, h:h + 1], None, op0=OP.mult)
        for c in range(NCORES):
            dma3[c % 3].dma_start(
                a2a_ins[h][c * A2A:(c + 1) * A2A]
                .rearrange("(p t) -> p t", p=128),
                ctxT[h][:, c * TOK:(c + 1) * TOK])
        nc.gpsimd.collective_compute("AllToAll", OP.bypass,
                                     ins=[a2a_ins[h][:]],
                                     outs=[a2a_outs[h][:]],
                                     replica_groups=rg)
    if DEBUG:
        nc.sync.dma_start(g["dbg_ctxT"][:], ctxT[0][:])
        dbv = p7.tile([128, CH], F32, tag="dbv")
        nc.vector.tensor_copy(dbv[:], vqi[:, 0:CH])
        nc.sync.dma_start(g["dbg_vqi"][:], dbv[:])
        nc.sync.dma_start(g["dbg_k2"][:], k2T[0][:].bitcast(F32))
    ps7z_cm.__exit__(None, None, None)
    ps7c_cm.__exit__(None, None, None)
    ps7s_cm.__exit__(None, None, None)
    p7m_cm.__exit__(None, None, None)
    p7_cm.__exit__(None, None, None)
    ctx_cm.__exit__(None, None, None)

    # -------- P9: token-sharded output projection --------
    p9_cm, p9 = pool("p9", bufs=2)
    ps9t_cm, ps9t = pool("ps9t", bufs=2, space="PSUM")
    cb = p9.tile([128, NE * TOK], F32, tag="cb", bufs=1)
    for b in range(NE):
        c, h = b // 2, b % 2
        dma3[b % 3].dma_start(
            cb[:, b * TOK:(b + 1) * TOK],
            a2a_outs[h][c * A2A:(c + 1) * A2A]
            .rearrange("(p t) -> p t", p=128))
    cab = p9.tile([128, NE * TOK], F32, tag="cab", bufs=1)
    for b in range(NE):
        bsl = slice(b * TOK, (b + 1) * TOK)
        nc.scalar.activation(cab[:, bsl], cb[:, bsl], ACTF.Abs)
    cmf = p9.tile([128, TOK], F32, tag="cmf", bufs=1)
    nc.vector.tensor_tensor(cmf[:], cab[:, 0:TOK], cab[:, TOK:2 * TOK],
                            op=OP.max)
    for b in range(2, NE):
        nc.vector.tensor_tensor(cmf[:], cmf[:],
                                cab[:, b * TOK:(b + 1) * TOK], op=OP.max)
    cmax_t = p9.tile([128, 2], F32, tag="cmaxt", bufs=1)
    for half in range(2):
        psT = ps9t.tile([128, 128], F32, tag="pst9", name=f"pst9_{half}")
        nc.tensor.transpose(psT[:], cmf[:, half * 128:(half + 1) * 128],
                            ident[:].bitcast(F32))
        nc.vector.tensor_reduce(cmax_t[:, half:half + 1], psT[:], axis=AX.X,
                                op=OP.max, apply_absolute_value=True)
    sc_pp2 = p9.tile([128, 2], F32, tag="scpp2", bufs=1)
    nc.vector.tensor_scalar(sc_pp2[:], cmax_t[:], EPS, 1.0 / Q8,
                            op0=OP.max, op1=OP.mult)
    rsc_pp2 = p9.tile([128, 2], F32, tag="rscpp2", bufs=1)
    nc.vector.reciprocal(rsc_pp2[:], sc_pp2[:])
    nc.sync.dma_start(g["scb"].rearrange("(j p) -> p j", p=128), sc_pp2[:])
    nc.sync.dma_start(g["rscb"].rearrange("(j p) -> p j", p=128), rsc_pp2[:])
    sc_rep = p9.tile([128, TOK], F32, tag="screp", bufs=1)
    nc.sync.dma_start(sc_rep[:],
                      g["scb"].rearrange("(a t) -> a t", a=1)
                      .to_broadcast([128, TOK]))
    rsc_full = p9.tile([128, NE * TOK], F32, tag="rscfull", bufs=1)
    for b in range(NE):
        dma3[b % 3].dma_start(
            rsc_full[:, b * TOK:(b + 1) * TOK],
            g["rscb"].rearrange("(a t) -> a t", a=1)
            .to_broadcast([128, TOK]))
    ps9t_cm.__exit__(None, None, None)
    ps9_cm, ps9 = pool("ps9", space="PSUM")
    ci = p9.tile([128, NE * TOK], BF16, tag="ci", bufs=1)
    for hb in range(2):
        bsl = slice(hb * 8 * TOK, (hb + 1) * 8 * TOK)
        t1 = p9.tile([128, 8 * TOK], F32, tag="t1", bufs=2)
        nc.vector.tensor_tensor(t1[:], cb[:, bsl], rsc_full[:, bsl],
                                op=OP.mult)
        t2 = p9.tile([128, 8 * TOK], F32, tag="t2", bufs=2)
        nc.scalar.activation(t2[:], t1[:], ACTF.Identity, bias=rndc_pp[:])
        nc.vector.tensor_scalar(ci[:, bsl], t2[:], RND_C, None,
                                op0=OP.subtract)
    if DEBUG:
        nc.sync.dma_start(g["dbg_cb"][:], cb[:])
        dbi = p9.tile([128, NE * TOK], F32, tag="dbi")
        nc.vector.tensor_copy(dbi[:], ci[:])
        nc.sync.dma_start(g["dbg_ci"][:], dbi[:])
    for ph in range(2):
        pO = [ps9.tile([128, TOK], F32, tag=f"pO{q}", name=f"pO{ph}_{q}")
              for q in range(8)]
        for b in range(NE):
            wo_et = p9.tile([128, E // 2], BF16, tag="woet", bufs=3)
            dma3[b % 3].dma_start(
                wo_et[:], g["wo_d"][b * 128:(b + 1) * 128,
                                    ph * 1024:(ph + 1) * 1024])
            for oo in range(8):
                nc.tensor.matmul(
                    pO[oo][:],
                    wo_et[:, oo * 128:(oo + 1) * 128],
                    ci[:, b * TOK:(b + 1) * TOK],
                    start=(b == 0), stop=(b == NE - 1))
        for oo in range(8):
            o = ph * 8 + oo
            f1 = p9.tile([128, TOK], F32, tag="f1", bufs=2)
            nc.scalar.activation(f1[:], pO[oo][:], ACTF.Identity,
                                 bias=0.0, scale=swo_pp[:, o:o + 1])
            f2 = p9.tile([128, TOK], F32, tag="f2", bufs=2)
            nc.vector.tensor_tensor(f2[:], f1[:], sc_rep[:], op=OP.mult)
            outsb = p9.tile([128, TOK], F32, tag="outsb", bufs=2)
            nc.vector.tensor_scalar(outsb[:], f2[:], ob_pp[:, o:o + 1], None,
                                    op0=OP.add)
            nc.sync.dma_start(g["outT_d"][o * 128:(o + 1) * 128, :],
                              outsb[:])
    ps9_cm.__exit__(None, None, None)
    p9_cm.__exit__(None, None, None)

    vq_cm.__exit__(None, None, None)
    kx_cm.__exit__(None, None, None)
    per_cm.__exit__(None, None, None)


# ==================== host side ====================

_CACHE = {}


def _get_nc(causal):
    if causal not in _CACHE:
        _CACHE[causal] = build(causal)
    return _CACHE[causal]


def _quant_w(w):
    amax = np.max(np.abs(w), axis=-1, keepdims=True)
    s = np.maximum(amax, np.float32(EPS)) / np.float32(Q8)
    wi = np.round((w / s).astype(np.float32))
    return wi, s[:, 0].astype(np.float32)


def kernel(hidden_states, attention_mask, q_w, q_b, k_w, k_b, v_w, v_b,
           o_w, o_b, num_heads):
    hidden_states = np.asarray(hidden_states, dtype=np.float32)
    attention_mask = np.asarray(attention_mask, dtype=np.float32)
    assert int(num_heads) == H
    B, T_, E_ = hidden_states.shape
    assert (B, T_, E_) == (1, T, E)

    x = np.ascontiguousarray(hidden_states[0])        # [T, E]

    causal_ref = np.triu(np.full((T, T), np.float32(NEG), np.float32), k=1)
    mfull = np.ascontiguousarray(attention_mask[0, 0])
    causal = bool(np.array_equal(mfull, causal_ref))

    nc = _get_nc(causal)

    wqi, sq = _quant_w(np.asarray(q_w, np.float32))
    wki, sk = _quant_w(np.asarray(k_w, np.float32))
    wvi, sv = _quant_w(np.asarray(v_w, np.float32))
    woi, so = _quant_w(np.asarray(o_w, np.float32))

    tblk = np.triu(np.full((128, 128), np.float32(NEG), np.float32), k=1)
    tblkT = np.ascontiguousarray(tblk.T)
    ident = np.eye(128, dtype=np.float32)
    identb = np.eye(128, dtype=np.float32).astype(ml_dtypes.bfloat16)
    rowvec = np.float32(T) - np.arange(T, dtype=np.float32)
    rvr = (np.float32(1.0) / rowvec).reshape(4, 512).astype(np.float32)

    woT = np.ascontiguousarray(woi.T).astype(ml_dtypes.bfloat16)  # [E(e),E(o)]
    swo_full = np.ascontiguousarray(so)
    ob_full = np.ascontiguousarray(np.asarray(o_b, np.float32))

    in_maps = []
    for c in range(NCORES):
        ch = slice(c * CH, (c + 1) * CH)
        tk = slice(c * TOK, (c + 1) * TOK)
        im = dict(
            xrows=np.ascontiguousarray(x[tk, :]),
            wq=np.ascontiguousarray(wqi[ch, :].T).astype(ml_dtypes.bfloat16),
            wk=np.ascontiguousarray(wki[ch, :].T).astype(ml_dtypes.bfloat16),
            wv=np.ascontiguousarray(wvi[ch, :].T).astype(ml_dtypes.bfloat16),
            wo=woT,
            swq=np.ascontiguousarray(sq[ch]),
            swk=np.ascontiguousarray(sk[ch]),
            swv=np.ascontiguousarray(sv[ch]),
            swo=swo_full,
            qb=np.ascontiguousarray(np.asarray(q_b, np.float32)[ch]),
            kb=np.ascontiguousarray(np.asarray(k_b, np.float32)[ch]),
            vb=np.ascontiguousarray(np.asarray(v_b, np.float32)[ch]),
            ob=ob_full,
            tblk=tblk, tblkT=tblkT, ident=ident, identb=identb, rvr=rvr,
        )
        if not causal:
            im["mask"] = mfull
            im["maskT"] = np.ascontiguousarray(mfull.T)
        in_maps.append(im)

    res = run_bass_kernel_spmd(nc, in_maps, list(range(NCORES)))
    kernel.last_results = res.results
    out = np.empty((T, E), dtype=np.float32)
    for c in range(NCORES):
        out[c * TOK:(c + 1) * TOK, :] = res.results[c]["outT"].T
    return out.reshape(1, T, E)


# revision 27
# speedup vs baseline: 1.0209x; 1.0209x over previous
"""CalScaleOPTAttention on 8 TRN2 NeuronCores (v2).

Sharding: heads across cores (2 heads / core, 256 channels each), with two
token-sharded cooperative stages to kill redundant DVE work:
  - x int8 quantization is sharded by tokens (each core quantizes its 256
    tokens with per-partition scales, PE-transposes, AllGathers the bf16 ints
    + fp32 scales in one collective).
  - the output projection is sharded by tokens via AllToAll: each core
    receives the full-E context slice for its 256 tokens, computes the
    per-token quant scale locally (no AllReduce), and o-projects with the
    full wo, streaming weight tiles from HBM.
Pass-2 attention computes S^T = K_mixed @ Q^T directly (scores with keys on
partitions), so exp(S^T) feeds the PV matmul with zero transposes; the
softmax denominator comes from a ones-row matmul and 1/z = exp(-ln z) on the
scalar engine (same activation table as Exp).

Collectives: AllGather(x ints + scales), AllReduce-max(k/v quant stats),
AllReduce-add(accumulated attention score, both heads), AllToAll(ctx) x2.
"""

import numpy as np
import ml_dtypes

import concourse.bass as bass
import concourse.mybir as mybir
import concourse.tile as tile
from concourse import bacc
from concourse.bass_utils import run_bass_kernel_spmd

F32 = mybir.dt.float32
F32R = mybir.dt.float32r
BF16 = mybir.dt.bfloat16
AX = mybir.AxisListType
OP = mybir.AluOpType
ACTF = mybir.ActivationFunctionType

NCORES = 8
T = 2048
E = 2048
H = 16
D = 128                   # head dim
HL = H // NCORES          # heads per core = 2
CH = HL * D               # channels per core = 256
TOK = T // NCORES         # tokens per core = 256
NT = T // 128             # 16 row tiles
NE = E // 128             # 16 contraction tiles
Q8 = 127.0
Q4 = 7.0
EPS = 1e-5
NEG = -1e9
RND_C = 12582912.0        # 1.5 * 2**23 round-to-int trick constant
SCALING = float(D) ** -0.5
K_TOP = T // 40           # 51
XBLK = 128 * TOK          # one e-tile block of x ints in the AG buffer
A2A = 128 * TOK           # per-shard ctx elements


def _cdiv(a, b):
    return (a + b - 1) // b


DEBUG = False


def build(causal: bool):
    nc = bacc.Bacc("TRN2", target_bir_lowering=False, debug=False,
                   num_devices=NCORES)

    def dt_in(n, s, d):
        return nc.dram_tensor(n, s, d, kind="ExternalInput").ap()

    g = {}
    g["xrows_d"] = dt_in("xrows", [TOK, E], F32)
    for w in ("wq", "wk", "wv"):
        g[w + "_d"] = dt_in(w, [E, CH], BF16)
    g["wo_d"] = dt_in("wo", [E, E], BF16)
    for v in ("swq", "swk", "swv", "qb", "kb", "vb"):
        g[v + "_d"] = dt_in(v, [CH], F32)
    g["swo_d"] = dt_in("swo", [E], F32)
    g["ob_d"] = dt_in("ob", [E], F32)
    g["tblk_d"] = dt_in("tblk", [128, 128], F32)
    g["tblkT_d"] = dt_in("tblkT", [128, 128], F32)
    g["ident_d"] = dt_in("ident", [128, 128], F32R)
    g["identb_d"] = dt_in("identb", [128, 128], BF16)
    g["rvr_d"] = dt_in("rvr", [4, 512], F32)
    if not causal:
        g["mask_d"] = dt_in("mask", [T, T], F32)
        g["maskT_d"] = dt_in("maskT", [T, T], F32)

    g["outT_d"] = nc.dram_tensor("outT", [E, TOK], F32,
                                 kind="ExternalOutput").ap()
    if DEBUG:
        for nm, shp in (("dbg_qT", [128, T]), ("dbg_kT", [128, T]),
                        ("dbg_yv", [128, CH]), ("dbg_acc", [1, T]),
                        ("dbg_ssel", [1, T]), ("dbg_ctxT", [128, T]),
                        ("dbg_xq", [128, T]), ("dbg_vqi", [128, CH]),
                        ("dbg_k2", [128, T]), ("dbg_cb", [128, NE * TOK]),
                        ("dbg_ci", [128, NE * TOK]), ("dbg_sx", [1, T])):
            g[nm] = nc.dram_tensor(nm, shp, F32, kind="ExternalOutput").ap()

    HXB = NE * XBLK // 2
    g["agxa_in"] = nc.dram_tensor("agxa_in", [HXB], BF16).ap()
    g["agxa_out"] = nc.dram_tensor("agxa_out", [NCORES, HXB], BF16,
                                   addr_space="Shared").ap()
    g["agxb_in"] = nc.dram_tensor("agxb_in", [HXB], BF16).ap()
    g["agxb_out"] = nc.dram_tensor("agxb_out", [NCORES, HXB], BF16,
                                   addr_space="Shared").ap()
    g["sx_in"] = nc.dram_tensor("sx_in", [TOK], F32).ap()
    g["sx_out"] = nc.dram_tensor("sx_out", [NCORES, TOK], F32,
                                 addr_space="Shared").ap()
    g["sxb"] = nc.dram_tensor("sxb", [T], F32).ap()
    g["kv_in"] = nc.dram_tensor("kv_in", [2, T], F32).ap()
    g["kv_out"] = nc.dram_tensor("kv_out", [2, T], F32,
                                 addr_space="Shared").ap()
    g["acc_in"] = nc.dram_tensor("acc_in", [2 * T], F32).ap()
    g["acc_out"] = nc.dram_tensor("acc_out", [2 * T], F32,
                                  addr_space="Shared").ap()
    g["flg_b"] = nc.dram_tensor("flg_b", [T], F32).ap()
    g["svb"] = nc.dram_tensor("svb", [CH], F32).ap()
    g["sselr_b"] = nc.dram_tensor("sselr_b", [T], F32).ap()
    g["rsselr_b"] = nc.dram_tensor("rsselr_b", [T], F32).ap()
    g["a2a0_in"] = nc.dram_tensor("a2a0_in", [NCORES * A2A], F32).ap()
    g["a2a0_out"] = nc.dram_tensor("a2a0_out", [NCORES * A2A], F32).ap()
    g["a2a1_in"] = nc.dram_tensor("a2a1_in", [NCORES * A2A], F32).ap()
    g["a2a1_out"] = nc.dram_tensor("a2a1_out", [NCORES * A2A], F32).ap()
    g["scb"] = nc.dram_tensor("scb", [TOK], F32).ap()
    g["rscb"] = nc.dram_tensor("rscb", [TOK], F32).ap()
    g["rg"] = [list(range(NCORES))]

    with tile.TileContext(nc) as tc:
        _body(nc, tc, causal, g)
    nc.compile()
    return nc


def _body(nc, tc, causal, g):
    rg = g["rg"]

    def pool(name, bufs=1, space="SBUF"):
        cm = tc.tile_pool(name=name, bufs=bufs, space=space)
        return cm, cm.__enter__()

    per_cm, per = pool("per")

    tblk = per.tile([128, 128], F32, tag="tblk")
    tblkT = per.tile([128, 128], F32, tag="tblkT")
    ident = per.tile([128, 128], F32R, tag="ident")
    identb = per.tile([128, 128], BF16, tag="identb")
    nc.sync.dma_start(tblk[:], g["tblk_d"][:])
    nc.sync.dma_start(tblkT[:], g["tblkT_d"][:])
    nc.sync.dma_start(ident[:], g["ident_d"][:])
    nc.sync.dma_start(identb[:], g["identb_d"][:])
    ones128 = per.tile([128, 128], BF16, tag="ones128")
    nc.vector.memset(ones128[:], 1.0)
    rndc_pp = per.tile([128, 1], F32, tag="rndcpp")
    nc.vector.memset(rndc_pp[:], RND_C)

    def load_pp(dram_vec, tag, ncol=2, p=per):
        t_ = p.tile([128, ncol], F32, tag=tag)
        nc.sync.dma_start(t_[:], dram_vec.rearrange("(j p) -> p j", p=128))
        return t_

    swq_pp = load_pp(g["swq_d"], "swq")
    swk_pp = load_pp(g["swk_d"], "swk")
    qb_pp = load_pp(g["qb_d"], "qb")
    kb_pp = load_pp(g["kb_d"], "kb")
    swo_pp = load_pp(g["swo_d"], "swo", ncol=NE)
    ob_pp = load_pp(g["ob_d"], "ob", ncol=NE)
    qT = [per.tile([128, T], F32R, tag=f"qT{h}", name=f"qT{h}")
          for h in range(HL)]
    k2T = [per.tile([128, T], F32R, tag=f"k2T{h}", name=f"k2T{h}")
           for h in range(HL)]
    sv_pp = per.tile([128, HL], F32, tag="svpp")

    # long-lived mid tensors, stack-nested
    kx_cm, kxp = pool("kxp")
    kTx = [kxp.tile([128, T], F32, tag=f"kTx{h}", name=f"kTx{h}")
           for h in range(HL)]
    vq_cm, vqp = pool("vqp")
    vqi = vqp.tile([128, NT * CH], BF16, tag="vqi")
    yv_cm, yvp = pool("yvp")
    yv = yvp.tile([128, NT * CH], F32, tag="yv")

    k1r_cm, k1rp = pool("k1rp")
    k1r = [k1rp.tile([128, T], F32R, tag=f"k1r{h}", name=f"k1r{h}")
           for h in range(HL)]

    # -------- P0: local token-shard quantize of x, then one AllGather ------
    p0_cm, p0 = pool("p0")
    ps0_cm, ps0 = pool("ps0", bufs=4, space="PSUM")
    xr = [p0.tile([128, E], F32, tag=f"xr{j}", name=f"xr{j}") for j in range(2)]
    nc.sync.dma_start(xr[0][:], g["xrows_d"][0:128, :])
    nc.scalar.dma_start(xr[1][:], g["xrows_d"][128:256, :])
    am = p0.tile([128, 2], F32, tag="am")
    for j in range(2):
        nc.vector.tensor_reduce(am[:, j:j + 1], xr[j][:], axis=AX.X,
                                op=OP.max, apply_absolute_value=True)
    sxp = p0.tile([128, 2], F32, tag="sxp")
    nc.vector.tensor_scalar(sxp[:], am[:], EPS, 1.0 / Q8,
                            op0=OP.max, op1=OP.mult)
    rsxp = p0.tile([128, 2], F32, tag="rsxp")
    nc.vector.reciprocal(rsxp[:], sxp[:])
    xTi = p0.tile([128, NE * TOK], BF16, tag="xTi")
    for j in range(2):
        xs = p0.tile([128, E], F32, tag="xs", bufs=2)
        nc.scalar.activation(xs[:], xr[j][:], ACTF.Identity,
                             bias=rndc_pp[:], scale=rsxp[:, j:j + 1])
        xi = p0.tile([128, E], BF16, tag="xi", bufs=2)
        nc.vector.tensor_scalar(xi[:], xs[:], RND_C, None, op0=OP.subtract)
        for et in range(NE):
            psx = ps0.tile([128, 128], BF16, tag="psx",
                           name=f"psx{j}_{et}")
            nc.tensor.transpose(psx[:], xi[:, et * 128:(et + 1) * 128],
                                identb[:])
            dst = xTi[:, et * TOK + j * 128: et * TOK + (j + 1) * 128]
            if et % 2 == 0:
                nc.scalar.copy(dst, psx[:])
            else:
                nc.vector.tensor_copy(dst, psx[:])
    dma3 = [nc.sync, nc.scalar, nc.gpsimd]
    HXB = NE * XBLK // 2
    for et in range(NE):
        agt = g["agxa_in"] if et < 8 else g["agxb_in"]
        base = (et % 8) * XBLK
        dma3[et % 3].dma_start(
            agt[base:base + XBLK].rearrange("(p t) -> p t", p=128),
            xTi[:, et * TOK:(et + 1) * TOK])
    nc.sync.dma_start(g["sx_in"].rearrange("(j p) -> p j", p=128), sxp[:])
    nc.gpsimd.collective_compute("AllGather", OP.bypass,
                                 ins=[g["sx_in"][:]], outs=[g["sx_out"][:]],
                                 replica_groups=rg)
    nc.gpsimd.collective_compute("AllGather", OP.bypass,
                                 ins=[g["agxa_in"][:]],
                                 outs=[g["agxa_out"][:]],
                                 replica_groups=rg)
    nc.gpsimd.collective_compute("AllGather", OP.bypass,
                                 ins=[g["agxb_in"][:]],
                                 outs=[g["agxb_out"][:]],
                                 replica_groups=rg)
    ps0_cm.__exit__(None, None, None)
    p0_cm.__exit__(None, None, None)

    # -------- w pool: xq + per-token scale reps --------
    w_cm, wp = pool("wpool")
    swv_row = wp.tile([1, CH], F32, tag="swvr")
    vb_row = wp.tile([1, CH], F32, tag="vbr")
    nc.sync.dma_start(swv_row[:], g["swv_d"].rearrange("(a c) -> a c", a=1))
    nc.sync.dma_start(vb_row[:], g["vb_d"].rearrange("(a c) -> a c", a=1))
    swv_rep = wp.tile([128, CH], F32, tag="swvrep")
    vb_rep = wp.tile([128, CH], F32, tag="vbrep")
    nc.gpsimd.partition_broadcast(swv_rep[:], swv_row[:])
    nc.gpsimd.partition_broadcast(vb_rep[:], vb_row[:])

    # -------- P1a: assemble xq + scales from the AllGather --------
    xq = wp.tile([128, NE * T], BF16, tag="xq")
    for et in range(NE):
        agt = g["agxa_out"] if et < 8 else g["agxb_out"]
        base = (et % 8) * XBLK
        for c in range(NCORES):
            dma3[(et * NCORES + c) % 3].dma_start(
                xq[:, et * T + c * TOK: et * T + (c + 1) * TOK],
                agt[c, base:base + XBLK]
                .rearrange("(p t) -> p t", p=128))
    sx_pp = wp.tile([128, 2 * NCORES], F32, tag="sxpp")
    for c in range(NCORES):
        nc.scalar.dma_start(
            sx_pp[:, c * 2:(c + 1) * 2],
            g["sx_out"][c, :].rearrange("(j p) -> p j", p=128))
    # bounce sx to DRAM row, then broadcast-load [128, T]
    nc.sync.dma_start(g["sxb"].rearrange("(j p) -> p j", p=128), sx_pp[:])
    sx_rep = wp.tile([128, T], F32, tag="sxrep")
    nc.sync.dma_start(sx_rep[:],
                      g["sxb"].rearrange("(a t) -> a t", a=1)
                      .to_broadcast([128, T]))
    if DEBUG:
        dbx = wp.tile([128, T], F32, tag="dbx")
        nc.vector.tensor_copy(dbx[:], xq[:, 0:T])
        nc.sync.dma_start(g["dbg_xq"][:], dbx[:])
        nc.sync.dma_start(g["dbg_sx"][:], sx_rep[0:1, :])

    # -------- P1b: Q/K/V projections --------
    p1b_cm, p1b = pool("p1b", bufs=3)
    ps1_cm, ps1 = pool("ps1", space="PSUM")
    for name, wd, sw_pp, b_pp in (
            ("q", g["wq_d"], swq_pp, qb_pp),
            ("k", g["wk_d"], swk_pp, kb_pp)):
        pP = [ps1.tile([128, T], F32, tag=f"pP{o}", name=f"pP{o}_{name}")
              for o in range(2)]
        for et in range(NE):
            we = p1b.tile([128, CH], BF16, tag="wstream")
            nc.sync.dma_start(we[:], wd[et * 128:(et + 1) * 128, :])
            xq_e = xq[:, et * T:(et + 1) * T]
            for o in range(2):
                for n in range(4):
                    nsl = slice(n * 512, (n + 1) * 512)
                    nc.tensor.matmul(pP[o][:, nsl],
                                     we[:, o * 128:(o + 1) * 128],
                                     xq_e[:, nsl],
                                     start=(et == 0), stop=(et == NE - 1))
        for o in range(2):
            e1 = p1b.tile([128, T], F32, tag="ev1", bufs=2)
            nc.scalar.activation(e1[:], pP[o][:], ACTF.Identity,
                                 bias=0.0, scale=sw_pp[:, o:o + 1])
            nc.vector.tensor_tensor(e1[:], e1[:], sx_rep[:], op=OP.mult)
            if name == "q":
                nc.vector.tensor_scalar(qT[o][:], e1[:], b_pp[:, o:o + 1],
                                        SCALING, op0=OP.add, op1=OP.mult)
            else:
                nc.vector.tensor_scalar(kTx[o][:], e1[:], b_pp[:, o:o + 1],
                                        None, op0=OP.add)
                nc.vector.tensor_copy(k1r[o][:], kTx[o][:])
    ps1_cm.__exit__(None, None, None)

    # V pass: [t, ch] = xq_block.T @ wv_int
    ps1v_cm, ps1v = pool("ps1v", bufs=2, space="PSUM")
    wv_res = p1b.tile([128, NE * CH], BF16, tag="wvres", bufs=1)
    for et in range(NE):
        nc.sync.dma_start(wv_res[:, et * CH:(et + 1) * CH],
                          g["wv_d"][et * 128:(et + 1) * 128, :])
    for j in range(NT):
        pVj = ps1v.tile([128, CH], F32, tag="pV", name=f"pV{j}")
        for et in range(NE):
            xq_e = xq[:, et * T:(et + 1) * T]
            nc.tensor.matmul(pVj[:],
                             xq_e[:, j * 128:(j + 1) * 128],
                             wv_res[:, et * CH:(et + 1) * CH],
                             start=(et == 0), stop=(et == NE - 1))
        ysl = slice(j * CH, (j + 1) * CH)
        e3 = p1b.tile([128, CH], F32, tag="ev3")
        nc.scalar.activation(e3[:], pVj[:], ACTF.Identity,
                             bias=0.0, scale=sx_pp[:, j:j + 1])
        nc.vector.tensor_tensor(e3[:], e3[:], swv_rep[:], op=OP.mult)
        nc.vector.tensor_tensor(yv[:, ysl], e3[:], vb_rep[:], op=OP.add)
    ps1v_cm.__exit__(None, None, None)
    p1b_cm.__exit__(None, None, None)
    w_cm.__exit__(None, None, None)

    if DEBUG:
        nc.sync.dma_start(g["dbg_qT"][:], qT[0][:].bitcast(F32))
        nc.sync.dma_start(g["dbg_kT"][:], kTx[0][:])
        nc.sync.dma_start(g["dbg_yv"][:], yv[:, 0:CH])

    # -------- P23: k row-max + v out-quant row-max, AllReduce-max --------
    p23_cm, p23 = pool("p23", bufs=2)
    ps23_cm, ps23 = pool("ps23", bufs=2, space="PSUM")
    kmm = p23.tile([128, T], F32, tag="kmm", bufs=1)
    kab = p23.tile([128, T], F32, tag="kab", bufs=1)
    nc.scalar.activation(kmm[:], kTx[0][:], ACTF.Abs)
    nc.scalar.activation(kab[:], kTx[1][:], ACTF.Abs)
    nc.vector.tensor_tensor(kmm[:], kmm[:], kab[:], op=OP.max)
    kmax_pp = p23.tile([128, NT], F32, tag="kmaxpp", bufs=1)
    for b in range(NT):
        psT = ps23.tile([128, 128], F32, tag="pstat", name=f"psk{b}")
        nc.tensor.transpose(psT[:], kmm[:, b * 128:(b + 1) * 128],
                            ident[:].bitcast(F32))
        nc.vector.tensor_reduce(kmax_pp[:, b:b + 1], psT[:],
                                axis=AX.X, op=OP.max,
                                apply_absolute_value=True)
    nc.sync.dma_start(g["kv_in"][0, :].rearrange("(j p) -> p j", p=128),
                      kmax_pp[:])
    ym = p23.tile([128, NT], F32, tag="ym")
    for j in range(NT):
        nc.vector.tensor_reduce(ym[:, j:j + 1], yv[:, j * CH:(j + 1) * CH],
                                axis=AX.X, op=OP.max,
                                apply_absolute_value=True)
    nc.sync.dma_start(g["kv_in"][1, :].rearrange("(j p) -> p j", p=128),
                      ym[:])
    nc.gpsimd.collective_compute("AllReduce", OP.max,
                                 ins=[g["kv_in"][:]], outs=[g["kv_out"][:]],
                                 replica_groups=rg)
    ps23_cm.__exit__(None, None, None)
    p23_cm.__exit__(None, None, None)

    # -------- P5: pass 1 -> accumulated attention score --------
    p5_cm, p5 = pool("p5", bufs=3)
    p5m_cm, p5m = pool("p5m", bufs=2)
    psA_cm, psA_p = pool("psA", space="PSUM")
    psS_cm, psS_p = pool("psS", bufs=2, space="PSUM")
    pA = [psA_p.tile([1, 512], F32, tag=f"pA{n}", name=f"pA{n}")
          for n in range(4)]
    accs2 = [p5.tile([1, T], F32, tag=f"accs2_{h}", bufs=1,
                     name=f"accs2_{h}") for h in range(HL)]
    for h in range(HL):
        first_wr = [True] * 4
        zgA = p5.tile([128, 4], F32, tag="zgA", bufs=2, name=f"zgA{h}")
        zgB = p5.tile([128, 4], F32, tag="zgB", bufs=2, name=f"zgB{h}")
        p1ts = {}
        for i in range(NT):
            c_cols = (i + 1) * 128 if causal else T
            nh = 1 if c_cols <= 1024 else 2
            ig = i % 4
            if not causal:
                mrow = p5m.tile([128, T], F32, tag="mrow")
                nc.sync.dma_start(mrow[:],
                                  g["mask_d"][i * 128:(i + 1) * 128, :])
            p1t = p5.tile([128, T], BF16, tag="p1t", bufs=4,
                          name=f"p1t_{h}_{i}")
            p1ts[i] = p1t
            for half in range(nh):
                base = half * 1024
                hw = min(1024, c_cols - base)
                psS = psS_p.tile([128, 1024], F32, tag="pS",
                                 name=f"pS_{h}_{i}_{half}")
                for n in range(_cdiv(hw, 512)):
                    w = min(512, hw - n * 512)
                    nc.tensor.matmul(
                        psS[:, n * 512:n * 512 + w],
                        qT[h][:, i * 128:(i + 1) * 128],
                        k1r[h][:, base + n * 512:base + n * 512 + w],
                        start=True, stop=True)
                if causal and base <= i * 128 < base + 1024:
                    doff = i * 128 - base
                    nc.vector.tensor_tensor(psS[:, doff:doff + 128],
                                            psS[:, doff:doff + 128],
                                            tblk[:], op=OP.add)
                elif not causal:
                    nc.vector.tensor_tensor(psS[:, :hw], psS[:, :hw],
                                            mrow[:, base:base + hw],
                                            op=OP.add)
                ztgt = (zgA if half == 0 else zgB)[:, ig:ig + 1]
                nc.scalar.activation(p1t[:, base:base + hw], psS[:, :hw],
                                     ACTF.Exp, bias=0.0, scale=1.0,
                                     accum_out=ztgt)
            if causal and c_cols % 512 != 0:
                pad = _cdiv(c_cols, 512) * 512
                nc.vector.memset(p1t[:, c_cols:pad], 0.0)
            if ig == 3:
                gi0 = i - 3
                if c_cols > 1024:
                    zs = p5.tile([128, 4], F32, tag="zs", bufs=2,
                                 name=f"zs{h}_{i}")
                    nc.vector.tensor_tensor(zs[:], zgA[:], zgB[:], op=OP.add)
                else:
                    zs = zgA
                rz4 = p5.tile([128, 4], BF16, tag="rz4", bufs=2,
                              name=f"rz4_{h}_{i}")
                with nc.allow_low_precision(reason="fp32r matmul feed"):
                    nc.vector.reciprocal(rz4[:], zs[:])
                for ii in range(gi0, i + 1):
                    cc = (ii + 1) * 128 if causal else T
                    for n in range(_cdiv(cc, 512)):
                        nc.tensor.matmul(
                            pA[n][:], rz4[:, ii - gi0:ii - gi0 + 1],
                            p1ts[ii][:, n * 512:(n + 1) * 512],
                            start=first_wr[n], stop=(ii == NT - 1))
                        first_wr[n] = False
                p1ts = {}
        for n in range(4):
            if n % 2 == 0:
                nc.scalar.copy(accs2[h][:, n * 512:(n + 1) * 512], pA[n][:])
            else:
                nc.vector.tensor_copy(accs2[h][:, n * 512:(n + 1) * 512],
                                      pA[n][:])
    for h in range(HL):
        nc.sync.dma_start(
            g["acc_in"][h * T:(h + 1) * T].rearrange("(a t) -> a t", a=1),
            accs2[h][:])
    nc.gpsimd.collective_compute("AllReduce", OP.add,
                                 ins=[g["acc_in"][:]], outs=[g["acc_out"][:]],
                                 replica_groups=rg)
    psS_cm.__exit__(None, None, None)
    psA_cm.__exit__(None, None, None)
    p5m_cm.__exit__(None, None, None)
    p5_cm.__exit__(None, None, None)

    k1r_cm.__exit__(None, None, None)

    # -------- P4: v quantization --------
    p4_cm, p4 = pool("p4", bufs=2)
    vq1 = p4.tile([128, NT * CH], F32, tag="vq1")
    ymf = p4.tile([128, NT], F32, tag="ymf")
    nc.sync.dma_start(ymf[:], g["kv_out"][1, :].rearrange("(j p) -> p j",
                                                          p=128))
    sy = p4.tile([128, NT], F32, tag="sy")
    nc.vector.tensor_scalar(sy[:], ymf[:], EPS, 1.0 / Q8,
                            op0=OP.max, op1=OP.mult)
    rsy = p4.tile([128, NT], F32, tag="rsy")
    nc.vector.reciprocal(rsy[:], sy[:])
    for j in range(NT):
        ysl = slice(j * CH, (j + 1) * CH)
        d1 = p4.tile([128, CH], F32, tag="d1")
        nc.scalar.activation(d1[:], yv[:, ysl], ACTF.Identity,
                             bias=rndc_pp[:], scale=rsy[:, j:j + 1])
        nc.vector.tensor_scalar(vq1[:, ysl], d1[:], RND_C, sy[:, j:j + 1],
                                op0=OP.subtract, op1=OP.mult)
    ps4_cm, ps4 = pool("ps4", bufs=2, space="PSUM")
    vmax_pp = p4.tile([128, HL], F32, tag="vmaxpp", bufs=1)
    vab = p4.tile([128, NT * CH], F32, tag="vab", bufs=1)
    for j in range(NT):
        ysl = slice(j * CH, (j + 1) * CH)
        nc.scalar.activation(vab[:, ysl], vq1[:, ysl], ACTF.Abs)
    for ob in range(HL):
        vmm = p4.tile([128, 128], F32, tag="vmm", bufs=2, name=f"vmm{ob}")
        nc.vector.tensor_copy(vmm[:], vab[:, ob * 128:(ob + 1) * 128])
        for j in range(1, NT):
            nc.vector.tensor_tensor(
                vmm[:], vmm[:],
                vab[:, j * CH + ob * 128:j * CH + (ob + 1) * 128],
                op=OP.max)
        psT = ps4.tile([128, 128], F32, tag="pstat4", name=f"psv{ob}")
        nc.tensor.transpose(psT[:], vmm[:], ident[:].bitcast(F32))
        nc.vector.tensor_reduce(vmax_pp[:, ob:ob + 1], psT[:], axis=AX.X,
                                op=OP.max, apply_absolute_value=True)
    ps4_cm.__exit__(None, None, None)
    nc.vector.tensor_scalar(sv_pp[:], vmax_pp[:], EPS, 1.0 / Q8,
                            op0=OP.max, op1=OP.mult)
    rsv_pp = p4.tile([128, HL], F32, tag="rsvpp")
    nc.vector.reciprocal(rsv_pp[:], sv_pp[:])
    nc.sync.dma_start(g["svb"].rearrange("(h p) -> p h", p=128), rsv_pp[:])
    rsv_rep = p4.tile([128, CH], F32, tag="rsvrep")
    nc.sync.dma_start(rsv_rep[:],
                      g["svb"].rearrange("(a c) -> a c", a=1)
                      .to_broadcast([128, CH]))
    for j in range(NT):
        ysl = slice(j * CH, (j + 1) * CH)
        m1 = p4.tile([128, CH], F32, tag="m1")
        nc.vector.tensor_tensor(m1[:], vq1[:, ysl], rsv_rep[:], op=OP.mult)
        m2 = p4.tile([128, CH], F32, tag="m2")
        nc.scalar.activation(m2[:], m1[:], ACTF.Identity, bias=rndc_pp[:])
        nc.vector.tensor_scalar(vqi[:, ysl], m2[:], RND_C, None,
                                op0=OP.subtract)
    p4_cm.__exit__(None, None, None)
    yv_cm.__exit__(None, None, None)

    # -------- P6: top-k flags + k scale selection + k2 ints --------
    p6_cm, p6 = pool("p6")
    acc4 = p6.tile([4, 512], F32, tag="acc4")
    acc4b = p6.tile([4, 512], F32, tag="acc4b")
    nc.sync.dma_start(acc4[:],
                      g["acc_out"][0:T].rearrange("(r s) -> r s", r=4))
    nc.sync.dma_start(acc4b[:],
                      g["acc_out"][T:2 * T].rearrange("(r s) -> r s", r=4))
    nc.vector.tensor_tensor(acc4[:], acc4[:], acc4b[:], op=OP.add)
    if DEBUG:
        nc.sync.dma_start(g["dbg_acc"].rearrange("a (r s) -> (a r) s", r=4),
                          acc4[:])
    rvr = p6.tile([4, 512], F32, tag="rvr")
    nc.sync.dma_start(rvr[:], g["rvr_d"][:])
    nc.vector.tensor_tensor(acc4[:], acc4[:], rvr[:], op=OP.mult)
    nc.vector.tensor_scalar(acc4[:], acc4[:], 1.0 / H, None, op0=OP.mult)
    tkw = p6.tile([4, 512], F32, tag="tkw")
    ton = acc4[:]
    for k_on in range(0, K_TOP, 8):
        k_this = min(k_on + 8, K_TOP) - k_on
        mx8 = p6.tile([4, 8], F32, tag="mx8")
        nc.vector.max(out=mx8[:], in_=ton)
        if k_this < 8:
            nc.vector.memset(mx8[:, k_this:], 0)
        nc.vector.match_replace(out=tkw[:], in_to_replace=mx8[:],
                                in_values=ton, imm_value=0)
        ton = tkw[:]
    nc.vector.tensor_sub(out=tkw[:], in0=acc4[:], in1=tkw[:])
    nc.vector.tensor_scalar(tkw[:], tkw[:], 0.0, None, op0=OP.is_gt)
    nc.sync.dma_start(g["flg_b"].rearrange("(r s) -> r s", r=4), tkw[:])
    flags = p6.tile([1, T], F32, tag="flags")
    nc.sync.dma_start(flags[:], g["flg_b"].rearrange("(a t) -> a t", a=1))
    flags_i = p6.tile([1, T], mybir.dt.int32, tag="flagsi")
    nc.vector.tensor_scalar(flags_i[:], flags[:], 0.0, None, op0=OP.is_gt)

    kmaxf = p6.tile([1, T], F32, tag="kmaxf")
    nc.sync.dma_start(kmaxf[:], g["kv_out"][0:1, :])
    s8 = p6.tile([1, T], F32, tag="s8")
    nc.vector.tensor_scalar(s8[:], kmaxf[:], 1.0 / Q8, EPS,
                            op0=OP.mult, op1=OP.max)
    ssel = p6.tile([1, T], F32, tag="ssel")
    nc.vector.tensor_scalar(ssel[:], kmaxf[:], EPS, 1.0 / Q4,
                            op0=OP.max, op1=OP.mult)
    nc.vector.copy_predicated(ssel[:], flags_i[:], s8[:])
    rssel = p6.tile([1, T], F32, tag="rssel")
    nc.vector.reciprocal(rssel[:], ssel[:])
    ssel_rep = p6.tile([128, T], F32, tag="sselrep")
    rssel_rep = p6.tile([128, T], F32, tag="rsselrep")
    nc.sync.dma_start(g["sselr_b"].rearrange("(a t) -> a t", a=1), ssel[:])
    nc.sync.dma_start(g["rsselr_b"].rearrange("(a t) -> a t", a=1), rssel[:])
    nc.sync.dma_start(ssel_rep[:],
                      g["sselr_b"].rearrange("(a t) -> a t", a=1)
                      .to_broadcast([128, T]))
    nc.sync.dma_start(rssel_rep[:],
                      g["rsselr_b"].rearrange("(a t) -> a t", a=1)
                      .to_broadcast([128, T]))
    if DEBUG:
        nc.sync.dma_start(g["dbg_ssel"][:], ssel[:])
    for h in range(HL):
        kq = p6.tile([128, T], F32, tag="kq")
        nc.vector.tensor_tensor(kq[:], kTx[h][:], rssel_rep[:], op=OP.mult)
        kq2 = p6.tile([128, T], F32, tag="kq2")
        nc.scalar.activation(kq2[:], kq[:], ACTF.Identity, bias=rndc_pp[:])
        nc.vector.tensor_scalar(kq2[:], kq2[:], RND_C, None, op0=OP.subtract)
        nc.vector.tensor_tensor(k2T[h][:], kq2[:], ssel_rep[:], op=OP.mult)
    p6_cm.__exit__(None, None, None)

    # -------- P7: pass 2 attention, S^T layout --------
    ctx_cm, ctxp = pool("ctxp")
    ctxT = [ctxp.tile([128, T], F32, tag=f"ctxT{h}", name=f"ctxT{h}")
            for h in range(HL)]
    p7_cm, p7 = pool("p7", bufs=2)
    p7m_cm, p7m = pool("p7m", bufs=2)
    ps7s_cm, ps7S = pool("ps7s", space="PSUM")
    ps7c_cm, ps7C = pool("ps7c", bufs=2, space="PSUM")
    ps7z_cm, ps7Z = pool("ps7z", bufs=2, space="PSUM")
    a2a_ins = [g["a2a0_in"], g["a2a1_in"]]
    a2a_outs = [g["a2a0_out"], g["a2a1_out"]]
    for h in range(HL):
        for n in range(4):
            tbase = n * 512
            jmax = 4 * n + 3 if causal else NT - 1
            psC = ps7C.tile([128, 512], F32, tag="pC", name=f"pC{h}_{n}")
            psZ = ps7Z.tile([128, 512], F32, tag="pZ", name=f"pZ{h}_{n}")
            for jg in range(0, jmax + 1, 2):
                jn = min(2, jmax + 1 - jg)
                psSg = ps7S.tile([128, 1024], F32, tag="pSg", bufs=2,
                                 name=f"pSg{h}_{n}_{jg}")
                offs = []
                for jj in range(jn):
                    j = jg + jj
                    off = max(0, j * 128 - tbase) if causal else 0
                    offs.append(off)
                    if off > 0:
                        nc.vector.memset(psSg[:, jj * 512: jj * 512 + off],
                                         NEG)
                    nc.tensor.matmul(
                        psSg[:, jj * 512 + off: (jj + 1) * 512],
                        k2T[h][:, j * 128:(j + 1) * 128],
                        qT[h][:, tbase + off: tbase + 512],
                        start=True, stop=True)
                    if causal and tbase <= j * 128 < tbase + 512:
                        dsl = slice(jj * 512 + off, jj * 512 + off + 128)
                        nc.vector.tensor_tensor(psSg[:, dsl], psSg[:, dsl],
                                                tblkT[:], op=OP.add)
                    elif not causal:
                        mrowT = p7m.tile([128, 512], F32, tag="mrowT")
                        nc.sync.dma_start(
                            mrowT[:],
                            g["maskT_d"][j * 128:(j + 1) * 128,
                                         tbase:tbase + 512])
                        nc.vector.tensor_tensor(
                            psSg[:, jj * 512:(jj + 1) * 512],
                            psSg[:, jj * 512:(jj + 1) * 512],
                            mrowT[:], op=OP.add)
                expg = p7.tile([128, 1024], BF16, tag="expg", bufs=3,
                               name=f"expg{h}_{n}_{jg}")
                nc.scalar.activation(expg[:, :jn * 512], psSg[:, :jn * 512],
                                     ACTF.Exp, bias=0.0, scale=1.0)
                for jj in range(jn):
                    j = jg + jj
                    nc.tensor.matmul(
                        psC[:],
                        vqi[:, j * CH + h * 128: j * CH + (h + 1) * 128],
                        expg[:, jj * 512:(jj + 1) * 512],
                        start=(j == 0), stop=(j == jmax))
                for jj in range(jn):
                    j = jg + jj
                    nc.tensor.matmul(
                        psZ[:], ones128[:],
                        expg[:, jj * 512:(jj + 1) * 512],
                        start=(j == 0), stop=(j == jmax))
            rzr = p7.tile([128, 512], F32, tag="rzr")
            nc.vector.reciprocal(rzr[:], psZ[:])
            cx = p7.tile([128, 512], F32, tag="cx")
            nc.vector.tensor_tensor(cx[:], psC[:], rzr[:], op=OP.mult)
            nc.vector.tensor_scalar(ctxT[h][:, tbase:tbase + 512], cx[:],
                                    sv_pp[:, h:h + 1], None, op0=OP.mult)
        for c in range(NCORES):
            dma3[c % 3].dma_start(
                a2a_ins[h][c * A2A:(c + 1) * A2A]
                .rearrange("(p t) -> p t", p=128),
                ctxT[h][:, c * TOK:(c + 1) * TOK])
        nc.gpsimd.collective_compute("AllToAll", OP.bypass,
                                     ins=[a2a_ins[h][:]],
                                     outs=[a2a_outs[h][:]],
                                     replica_groups=rg)
    if DEBUG:
        nc.sync.dma_start(g["dbg_ctxT"][:], ctxT[0][:])
        dbv = p7.tile([128, CH], F32, tag="dbv")
        nc.vector.tensor_copy(dbv[:], vqi[:, 0:CH])
        nc.sync.dma_start(g["dbg_vqi"][:], dbv[:])
        nc.sync.dma_start(g["dbg_k2"][:], k2T[0][:].bitcast(F32))
    ps7z_cm.__exit__(None, None, None)
    ps7c_cm.__exit__(None, None, None)
    ps7s_cm.__exit__(None, None, None)
    p7m_cm.__exit__(None, None, None)
    p7_cm.__exit__(None, None, None)
    ctx_cm.__exit__(None, None, None)

    # -------- P9: token-sharded output projection --------
    p9_cm, p9 = pool("p9", bufs=2)
    ps9t_cm, ps9t = pool("ps9t", bufs=2, space="PSUM")
    cb = p9.tile([128, NE * TOK], F32, tag="cb", bufs=1)
    for b in range(NE):
        c, h = b // 2, b % 2
        dma3[b % 3].dma_start(
            cb[:, b * TOK:(b + 1) * TOK],
            a2a_outs[h][c * A2A:(c + 1) * A2A]
            .rearrange("(p t) -> p t", p=128))
    cab = p9.tile([128, NE * TOK], F32, tag="cab", bufs=1)
    for b in range(NE):
        bsl = slice(b * TOK, (b + 1) * TOK)
        nc.scalar.activation(cab[:, bsl], cb[:, bsl], ACTF.Abs)
    cmf = p9.tile([128, TOK], F32, tag="cmf", bufs=1)
    nc.vector.tensor_tensor(cmf[:], cab[:, 0:TOK], cab[:, TOK:2 * TOK],
                            op=OP.max)
    for b in range(2, NE):
        nc.vector.tensor_tensor(cmf[:], cmf[:],
                                cab[:, b * TOK:(b + 1) * TOK], op=OP.max)
    cmax_t = p9.tile([128, 2], F32, tag="cmaxt", bufs=1)
    for half in range(2):
        psT = ps9t.tile([128, 128], F32, tag="pst9", name=f"pst9_{half}")
        nc.tensor.transpose(psT[:], cmf[:, half * 128:(half + 1) * 128],
                            ident[:].bitcast(F32))
        nc.vector.tensor_reduce(cmax_t[:, half:half + 1], psT[:], axis=AX.X,
                                op=OP.max, apply_absolute_value=True)
    sc_pp2 = p9.tile([128, 2], F32, tag="scpp2", bufs=1)
    nc.vector.tensor_scalar(sc_pp2[:], cmax_t[:], EPS, 1.0 / Q8,
                            op0=OP.max, op1=OP.mult)
    rsc_pp2 = p9.tile([128, 2], F32, tag="rscpp2", bufs=1)
    nc.vector.reciprocal(rsc_pp2[:], sc_pp2[:])
    nc.sync.dma_start(g["scb"].rearrange("(j p) -> p j", p=128), sc_pp2[:])
    nc.sync.dma_start(g["rscb"].rearrange("(j p) -> p j", p=128), rsc_pp2[:])
    sc_rep = p9.tile([128, TOK], F32, tag="screp", bufs=1)
    nc.sync.dma_start(sc_rep[:],
                      g["scb"].rearrange("(a t) -> a t", a=1)
                      .to_broadcast([128, TOK]))
    rsc_full = p9.tile([128, NE * TOK], F32, tag="rscfull", bufs=1)
    for b in range(NE):
        dma3[b % 3].dma_start(
            rsc_full[:, b * TOK:(b + 1) * TOK],
            g["rscb"].rearrange("(a t) -> a t", a=1)
            .to_broadcast([128, TOK]))
    ps9t_cm.__exit__(None, None, None)
    ps9_cm, ps9 = pool("ps9", space="PSUM")
    ci = p9.tile([128, NE * TOK], BF16, tag="ci", bufs=1)
    for hb in range(2):
        bsl = slice(hb * 8 * TOK, (hb + 1) * 8 * TOK)
        t1 = p9.tile([128, 8 * TOK], F32, tag="t1", bufs=2)
        nc.vector.tensor_tensor(t1[:], cb[:, bsl], rsc_full[:, bsl],
                                op=OP.mult)
        t2 = p9.tile([128, 8 * TOK], F32, tag="t2", bufs=2)
        nc.scalar.activation(t2[:], t1[:], ACTF.Identity, bias=rndc_pp[:])
        nc.vector.tensor_scalar(ci[:, bsl], t2[:], RND_C, None,
                                op0=OP.subtract)
    if DEBUG:
        nc.sync.dma_start(g["dbg_cb"][:], cb[:])
        dbi = p9.tile([128, NE * TOK], F32, tag="dbi")
        nc.vector.tensor_copy(dbi[:], ci[:])
        nc.sync.dma_start(g["dbg_ci"][:], dbi[:])
    for ph in range(2):
        pO = [ps9.tile([128, TOK], F32, tag=f"pO{q}", name=f"pO{ph}_{q}")
              for q in range(8)]
        for b in range(NE):
            wo_et = p9.tile([128, E // 2], BF16, tag="woet", bufs=3)
            dma3[b % 3].dma_start(
                wo_et[:], g["wo_d"][b * 128:(b + 1) * 128,
                                    ph * 1024:(ph + 1) * 1024])
            for oo in range(8):
                nc.tensor.matmul(
                    pO[oo][:],
                    wo_et[:, oo * 128:(oo + 1) * 128],
                    ci[:, b * TOK:(b + 1) * TOK],
                    start=(b == 0), stop=(b == NE - 1))
        for oo in range(8):
            o = ph * 8 + oo
            f1 = p9.tile([128, TOK], F32, tag="f1", bufs=2)
            nc.scalar.activation(f1[:], pO[oo][:], ACTF.Identity,
                                 bias=0.0, scale=swo_pp[:, o:o + 1])
            f2 = p9.tile([128, TOK], F32, tag="f2", bufs=2)
            nc.vector.tensor_tensor(f2[:], f1[:], sc_rep[:], op=OP.mult)
            outsb = p9.tile([128, TOK], F32, tag="outsb", bufs=2)
            nc.vector.tensor_scalar(outsb[:], f2[:], ob_pp[:, o:o + 1], None,
                                    op0=OP.add)
            nc.sync.dma_start(g["outT_d"][o * 128:(o + 1) * 128, :],
                              outsb[:])
    ps9_cm.__exit__(None, None, None)
    p9_cm.__exit__(None, None, None)

    vq_cm.__exit__(None, None, None)
    kx_cm.__exit__(None, None, None)
    per_cm.__exit__(None, None, None)


# ==================== host side ====================

_CACHE = {}


def _get_nc(causal):
    if causal not in _CACHE:
        _CACHE[causal] = build(causal)
    return _CACHE[causal]


def _quant_w(w):
    amax = np.max(np.abs(w), axis=-1, keepdims=True)
    s = np.maximum(amax, np.float32(EPS)) / np.float32(Q8)
    wi = np.round((w / s).astype(np.float32))
    return wi, s[:, 0].astype(np.float32)


def kernel(hidden_states, attention_mask, q_w, q_b, k_w, k_b, v_w, v_b,
           o_w, o_b, num_heads):
    hidden_states = np.asarray(hidden_states, dtype=np.float32)
    attention_mask = np.asarray(attention_mask, dtype=np.float32)
    assert int(num_heads) == H
    B, T_, E_ = hidden_states.shape
    assert (B, T_, E_) == (1, T, E)

    x = np.ascontiguousarray(hidden_states[0])        # [T, E]

    causal_ref = np.triu(np.full((T, T), np.float32(NEG), np.float32), k=1)
    mfull = np.ascontiguousarray(attention_mask[0, 0])
    causal = bool(np.array_equal(mfull, causal_ref))

    nc = _get_nc(causal)

    wqi, sq = _quant_w(np.asarray(q_w, np.float32))
    wki, sk = _quant_w(np.asarray(k_w, np.float32))
    wvi, sv = _quant_w(np.asarray(v_w, np.float32))
    woi, so = _quant_w(np.asarray(o_w, np.float32))

    tblk = np.triu(np.full((128, 128), np.float32(NEG), np.float32), k=1)
    tblkT = np.ascontiguousarray(tblk.T)
    ident = np.eye(128, dtype=np.float32)
    identb = np.eye(128, dtype=np.float32).astype(ml_dtypes.bfloat16)
    rowvec = np.float32(T) - np.arange(T, dtype=np.float32)
    rvr = (np.float32(1.0) / rowvec).reshape(4, 512).astype(np.float32)

    woT = np.ascontiguousarray(woi.T).astype(ml_dtypes.bfloat16)  # [E(e),E(o)]
    swo_full = np.ascontiguousarray(so)
    ob_full = np.ascontiguousarray(np.asarray(o_b, np.float32))

    in_maps = []
    for c in range(NCORES):
        ch = slice(c * CH, (c + 1) * CH)
        tk = slice(c * TOK, (c + 1) * TOK)
        im = dict(
            xrows=np.ascontiguousarray(x[tk, :]),
            wq=np.ascontiguousarray(wqi[ch, :].T).astype(ml_dtypes.bfloat16),
            wk=np.ascontiguousarray(wki[ch, :].T).astype(ml_dtypes.bfloat16),
            wv=np.ascontiguousarray(wvi[ch, :].T).astype(ml_dtypes.bfloat16),
            wo=woT,
            swq=np.ascontiguousarray(sq[ch]),
            swk=np.ascontiguousarray(sk[ch]),
            swv=np.ascontiguousarray(sv[ch]),
            swo=swo_full,
            qb=np.ascontiguousarray(np.asarray(q_b, np.float32)[ch]),
            kb=np.ascontiguousarray(np.asarray(k_b, np.float32)[ch]),
            vb=np.ascontiguousarray(np.asarray(v_b, np.float32)[ch]),
            ob=ob_full,
            tblk=tblk, tblkT=tblkT, ident=ident, identb=identb, rvr=rvr,
        )
        if not causal:
            im["mask"] = mfull
            im["maskT"] = np.ascontiguousarray(mfull.T)
        in_maps.append(im)

    res = run_bass_kernel_spmd(nc, in_maps, list(range(NCORES)))
    kernel.last_results = res.results
    out = np.empty((T, E), dtype=np.float32)
    for c in range(NCORES):
        out[c * TOK:(c + 1) * TOK, :] = res.results[c]["outT"].T
    return out.reshape(1, T, E)
